# revision 32
# baseline (speedup 1.0000x reference)
"""Trainium2 Bass kernel for ContractExpand (segment_reduce).

For each scale r in (1,2,4,10,25): segment-sum groups of r consecutive rows,
relu(Linear_r)/r, broadcast back to rows, concat all scales along rows.

Strategy: pure data parallel over 8 NeuronCores (row-sharded, 12500 rows per
core). The host passes x TRANSPOSED ([320, n] fp16; row 300 = ones, rows
301-319 = zero pad) with the r10/r25 segment sums appended per superchunk,
so input loads are large contiguous DMAs, the contraction dim is on
partitions, and the DVE only builds the r2/r4 sums on device. Output fp16
(tolerance 2e-2 rel); host upcasts.

Performance notes (all HW-measured):
  - Mixed-K matmul sequences cost ~235ns per PE row-config switch, so the
    45-row K-tail is zero-padded to a full 128 partitions (pad rows 64-127
    memzero'd once on device; rows 45-63 come zeroed from the host pad).
  - The PE reaches 2.4 GHz (HAM un-throttle) only when MMs stream without
    stalls; all non-MM work lives on the other engines.
  - DVE fp16 copy 0.53 ns/elem; ACT ~1 ns/elem; GpSimd ~3.5 ns/elem (bulk
    copies avoided there).
Engine assignment:
  - Scalar (ACT): input loads, r1 pair-relus, r4 pair-relus, r10 relus,
    r2 copies.
  - Vector (DVE): r2 segment reduce, r2 pair-relus, r25 relu, r4/r10/r25
    broadcast replication, one-time pad memzeros.
  - GpSimd: r4 segment sums (strided tensor_add), r10/r25 stores (SWDGE).
  - Sync: r1/r2/r4 stores.
Pair-relus cover two PSUM banks per instruction (strided 3D APs). r1 packs
8 tokens per partition, r2 4 groups x2, r4 2 groups x4 (4800B store
descriptors); ragged tails fall back to narrower packings.
"""

import sys

import numpy as np

if "/opt/trn_rl_repo" not in sys.path:
    sys.path.insert(0, "/opt/trn_rl_repo")

from contextlib import ExitStack

import concourse.tile as tile
from concourse import bacc, mybir

DIM = 300
KEXT = 320  # 300 dims + ones row + 19 zero-pad rows
SCALES = (1, 2, 4, 10, 25)
N_TOTAL = 100000
N_CORES = 8
R_CORE = N_TOTAL // N_CORES  # 12500
KSLICES = [(0, 128), (128, 256), (256, 320)]  # last: 45 real + 19 host-pad
F32 = mybir.dt.float32
DT = mybir.dt.float16
NPDT = np.float16
AXX = mybir.AxisListType.X


def _superchunks(rows):
    # small first chunk (fast pipeline ramp) and small last chunk (short
    # store-drain tail); %100 keeps every scale's groups chunk-aligned
    if rows == R_CORE:
        return [400, 3200, 3200, 3200, 1700, 800]
    out = []
    while rows > 0:
        sc = min(3200, rows)
        assert sc % 100 == 0
        out.append(sc)
        rows -= sc
    return out


def _emit(ctx, tc, xt_ap, wt_ap, out_ap, rows):
    nc = tc.nc

    singles = ctx.enter_context(tc.tile_pool(name="singles", bufs=1))
    pp2 = ctx.enter_context(tc.tile_pool(name="pp2", bufs=2, space="PSUM"))
    pp1 = ctx.enter_context(tc.tile_pool(name="pp1", bufs=4, space="PSUM"))
    h1pool = ctx.enter_context(tc.tile_pool(name="h1", bufs=8))
    h2pool = ctx.enter_context(tc.tile_pool(name="h2", bufs=8))
    h4pool = ctx.enter_context(tc.tile_pool(name="h4", bufs=6))
    h10pool = ctx.enter_context(tc.tile_pool(name="h10", bufs=4))
    h25pool = ctx.enter_context(tc.tile_pool(name="h25", bufs=2))

    # weights: one [128, 5*300] tile per k-slice (3 DMAs). The tail slice
    # is zero-padded to 128 partitions; rows 64-127 are never DMA'd, so
    # their memzero runs once with no write-write hazard.
    wt_sb = []
    for s, (k0, k1) in enumerate(KSLICES):
        t = singles.tile([128, len(SCALES) * DIM], DT, tag=f"wt_{s}")
        if k1 - k0 < 128:
            nc.vector.memzero(t[64:128, :])
        nc.sync.dma_start(out=t[: k1 - k0, :], in_=wt_ap[k0:k1, :])
        wt_sb.append(t)

    scs = _superchunks(rows)
    ccols = [sc + sc // 10 + sc // 25 for sc in scs]
    cbase = [sum(ccols[:i]) for i in range(len(scs))]
    max_cols = max(ccols)
    max_red = 1600 + 832  # r2 (pad 64) + r4

    xT = [
        [
            singles.tile([128, max_cols], DT, tag=f"xT{b}_{s}", name=f"xT{b}_{s}")
            for s in range(3)
        ]
        for b in range(2)
    ]
    tmpT = [
        [
            singles.tile([128, max_red], DT, tag=f"tm{b}_{s}", name=f"tm{b}_{s}")
            for s in range(3)
        ]
        for b in range(2)
    ]
    # zero pad partitions 64-127 of the tail-slice tiles once; DMA loads
    # and reduces only ever touch rows 0-63 there
    for b in range(2):
        nc.vector.memzero(xT[b][2][64:128, :])
        nc.vector.memzero(tmpT[b][2][64:128, :])

    def mm3(ps, srcs, base, cnt, si, step=1):
        for s in range(3):
            nc.tensor.matmul(
                ps[:cnt, :],
                srcs[s][:128, base : base + (cnt - 1) * step + 1 : step],
                wt_sb[s][:, si * DIM : (si + 1) * DIM],
                start=(s == 0),
                stop=(s == 2),
            )

    relu = mybir.ActivationFunctionType.Relu

    def load_chunk(ci):
        xbuf = xT[ci % 2]
        for s, (k0, k1) in enumerate(KSLICES):
            nc.scalar.dma_start(
                out=xbuf[s][: k1 - k0, : ccols[ci]],
                in_=xt_ap[k0:k1, cbase[ci] : cbase[ci] + ccols[ci]],
            )

    def emit_reduces(ci):
        # r2 sums on DVE, r4 (from r2) on GpSimd, into chunk ci's buffers
        xbuf = xT[ci % 2]
        tbuf = tmpT[ci % 2]
        sc_i = scs[ci]
        w2_i, w4_i = sc_i // 2, sc_i // 4
        lp = nc.allow_low_precision(reason="fp16 segment sums feed fp16 matmul")
        lp.__enter__()
        for s, (k0, k1) in enumerate(KSLICES):
            ksz = k1 - k0 if s < 2 else 64
            nc.vector.reduce_sum(
                out=tbuf[s][:ksz, :w2_i],
                in_=xbuf[s][:ksz, :sc_i].rearrange("p (g r) -> p g r", r=2),
                axis=AXX,
            )
        for s, (k0, k1) in enumerate(KSLICES):
            ksz = k1 - k0 if s < 2 else 64
            nc.gpsimd.tensor_add(
                out=tbuf[s][:ksz, 1600 : 1600 + w4_i],
                in0=tbuf[s][:ksz, 0 : w2_i - 1 : 2],
                in1=tbuf[s][:ksz, 1:w2_i:2],
            )
        lp.__exit__(None, None, None)

    load_chunk(0)
    row0 = 0
    for sci, sc in enumerate(scs):
        xb = xT[sci % 2]
        tb = tmpT[sci % 2]
        w2, w4 = sc // 2, sc // 4
        o10, o25 = sc, sc + sc // 10  # col offsets of host r10/r25 sums

        # prefetch next superchunk
        if sci + 1 < len(scs):
            load_chunk(sci + 1)
        emit_reduces(sci)

        # --- r=1: P tokens per partition (P=8 main, 4 tail); pair-relus on
        # ACT; one store per htile ---
        def emit_r1(c0, P):
            T = min(128 * P, sc - c0)
            M = T // P
            h = h1pool.tile([128, 4 * DIM], DT, tag="h1")
            for half in range(P // 2):
                pp = pp2.tile([128, 1024], F32, tag="pp2")
                for sub in range(2):
                    mm3(
                        pp[:M, 512 * sub : 512 * sub + DIM],
                        xb,
                        c0 + 2 * half + sub,
                        M,
                        0,
                        step=P,
                    )
                nc.scalar.activation(
                    out=h[:M, 2 * half * DIM : (2 * half + 2) * DIM],
                    in_=pp[:M, :].rearrange("p (two x) -> p two x", two=2)[
                        :, :, :DIM
                    ],
                    func=relu,
                )
            orow = row0 + c0
            nc.sync.dma_start(
                out=out_ap[orow : orow + T, :].rearrange(
                    "(p q) d -> p (q d)", q=P
                ),
                in_=h[:M, : P * DIM],
            )

        # --- r=2: G group-pairs per partition [A,A,B,B,...]; pair-relu DVE,
        # copy ACT ---
        def emit_r2(c0, G):
            T2 = min(128 * G, w2 - c0)
            M = T2 // G
            h = h2pool.tile([128, 4 * DIM], DT, tag="h2")
            for half in range(G // 2):
                pp = pp2.tile([128, 1024], F32, tag="pp2")
                for sub in range(2):
                    mm3(
                        pp[:M, 512 * sub : 512 * sub + DIM],
                        tb,
                        c0 + 2 * half + sub,
                        M,
                        1,
                        step=G,
                    )
                nc.vector.tensor_relu(
                    out=h[:M, : 2 * G * DIM].rearrange(
                        "p (g d2) -> p g d2", d2=2 * DIM
                    )[:, 2 * half : 2 * half + 2, :DIM],
                    in_=pp[:M, :].rearrange("p (two x) -> p two x", two=2)[
                        :, :, :DIM
                    ],
                )
            hv = h[:M, : 2 * G * DIM].rearrange("p (g d2) -> p g d2", d2=2 * DIM)
            nc.scalar.copy(out=hv[:, :, DIM:], in_=hv[:, :, :DIM])
            g0 = row0 // 2 + c0
            orow = rows + 2 * g0
            nc.sync.dma_start(
                out=out_ap[orow : orow + 2 * T2, :].rearrange(
                    "(p q) d -> p (q d)", q=2 * G
                ),
                in_=h[:M, : 2 * G * DIM],
            )

        # --- r=4: G groups per partition x4 rep; pair-relu ACT (G=2) or
        # single relu; broadcast DVE ---
        def emit_r4(c0, G):
            T4 = min(128 * G, w4 - c0)
            M = T4 // G
            h = h4pool.tile([128, 4 * DIM], DT, tag="h4")
            if G == 2:
                pp = pp2.tile([128, 1024], F32, tag="pp2")
                for sub in range(2):
                    mm3(
                        pp[:M, 512 * sub : 512 * sub + DIM],
                        tb,
                        1600 + c0 + sub,
                        M,
                        2,
                        step=2,
                    )
                nc.scalar.activation(
                    out=h[:M, :].rearrange("p (g d4) -> p g d4", d4=4 * DIM)[
                        :, :, :DIM
                    ],
                    in_=pp[:M, :].rearrange("p (two x) -> p two x", two=2)[
                        :, :, :DIM
                    ],
                    func=relu,
                )
            else:
                pp = pp1.tile([128, 512], F32, tag="pp1")
                mm3(pp[:M, :DIM], tb, 1600 + c0, M, 2)
                nc.scalar.activation(out=h[:M, :DIM], in_=pp[:M, :DIM], func=relu)
            hv = h[:M, : 4 * G * DIM].rearrange("p (g e d) -> p g e d", g=G, d=DIM)
            nc.vector.tensor_copy(
                out=hv[:, :, 1:, :],
                in_=hv[:, :, 0:1, :].broadcast_to([M, G, 3, DIM]),
            )
            g0 = row0 // 4 + c0
            orow = 2 * rows + 4 * g0
            nc.gpsimd.dma_start(
                out=out_ap[orow : orow + 4 * T4, :].rearrange(
                    "(p q) d -> p (q d)", q=4 * G
                ),
                in_=h[:M, : 4 * G * DIM],
            )

        # --- r=10: relu ACT, broadcast DVE, store GpSimd (SWDGE) ---
        def emit_r10(c0):
            M = min(128, sc // 10 - c0)
            pp = pp1.tile([128, 512], F32, tag="pp1")
            mm3(pp[:M, :DIM], xb, o10 + c0, M, 3)
            h = h10pool.tile([128, 10 * DIM], DT, tag="h10")
            nc.scalar.activation(out=h[:M, :DIM], in_=pp[:M, :DIM], func=relu)
            nc.vector.tensor_copy(
                out=h[:M, DIM:].rearrange("p (e d) -> p e d", d=DIM),
                in_=h[:M, :DIM].unsqueeze(1).broadcast_to([M, 9, DIM]),
            )
            g0 = row0 // 10 + c0
            orow = 3 * rows + 10 * g0
            nc.gpsimd.dma_start(
                out=out_ap[orow : orow + 10 * M, :].rearrange(
                    "(g e) d -> g (e d)", e=10
                ),
                in_=h[:M, :],
            )

        # --- r=25: relu + broadcast DVE, store GpSimd (SWDGE) ---
        def emit_r25(c0):
            M = min(128, sc // 25 - c0)
            pp = pp1.tile([128, 512], F32, tag="pp1")
            mm3(pp[:M, :DIM], xb, o25 + c0, M, 4)
            h = h25pool.tile([128, 25 * DIM], DT, tag="h25")
            nc.vector.tensor_relu(out=h[:M, :DIM], in_=pp[:M, :DIM])
            nc.vector.tensor_copy(
                out=h[:M, DIM:].rearrange("p (e d) -> p e d", d=DIM),
                in_=h[:M, :DIM].unsqueeze(1).broadcast_to([M, 24, DIM]),
            )
            g0 = row0 // 25 + c0
            orow = 4 * rows + 25 * g0
            nc.gpsimd.dma_start(
                out=out_ap[orow : orow + 25 * M, :].rearrange(
                    "(g e) d -> g (e d)", e=25
                ),
                in_=h[:M, :],
            )

        def spans(total, main):
            out = []
            c0 = 0
            while c0 < total:
                T = min(main, total - c0)
                out.append((c0, T))
                c0 += T
            return out

        # PE queue order: r1 tiles first (DMA-only deps), then r10/r25
        # (also DMA-only), then r2 (DVE dep), then r4 (GpSimd dep). The
        # r10/r25 tiles are interleaved among r1 to smooth store supply.
        big = [(emit_r25, c0) for c0 in range(0, sc // 25, 128)] + [
            (emit_r10, c0) for c0 in range(0, sc // 10, 128)
        ]
        bi = 0
        for c0, T in spans(sc, 512):
            emit_r1(c0, 4)
            if bi < len(big):
                fn, bc0 = big[bi]
                fn(bc0)
                bi += 1
        while bi < len(big):
            fn, bc0 = big[bi]
            fn(bc0)
            bi += 1
        for c0, T2 in spans(w2, 256):
            emit_r2(c0, 2)
        for c0, T4 in spans(w4, 128):
            emit_r4(c0, 1)

        row0 += sc


def build_nc(rows=R_CORE):
    nc = bacc.Bacc("TRN2", target_bir_lowering=False)
    scs = _superchunks(rows)
    tot_cols = sum(sc + sc // 10 + sc // 25 for sc in scs)
    xt = nc.declare_dram_parameter("xt", [KEXT, tot_cols], DT, isOutput=False)
    wt = nc.declare_dram_parameter(
        "wt", [KEXT, len(SCALES) * DIM], DT, isOutput=False
    )
    out = nc.declare_dram_parameter(
        "out", [len(SCALES) * rows, DIM], DT, isOutput=True
    )
    with tile.TileContext(nc) as tc:
        with ExitStack() as ctx:
            _emit(ctx, tc, xt.ap(), wt.ap(), out.ap(), rows)
    nc.compile()
    return nc


def make_wt(Ws, bs):
    """[320, 5*300]: column block i = [W_r.T / r ; b_r / r^2 ; zero pad]."""
    wt = np.zeros((KEXT, len(SCALES) * DIM), np.float32)
    for i, r in enumerate(SCALES):
        wt[:DIM, i * DIM : (i + 1) * DIM] = np.asarray(Ws[i], np.float32).T / r
        wt[DIM, i * DIM : (i + 1) * DIM] = np.asarray(bs[i], np.float32) / (r * r)
    return wt


def make_xt(x_shard):
    """[n,300] fp32 -> [320, tot_cols] fp16: per superchunk, transposed x
    (+ones row, +zero pad) followed by transposed r10 and r25 segment sums
    (+r row)."""
    n = len(x_shard)
    scs = _superchunks(n)
    tot_cols = sum(sc + sc // 10 + sc // 25 for sc in scs)
    xt = np.zeros((KEXT, tot_cols), NPDT)
    col = 0
    r0 = 0
    for sc in scs:
        xs = x_shard[r0 : r0 + sc]
        xt[:DIM, col : col + sc] = xs.astype(NPDT).T
        xt[DIM, col : col + sc] = 1.0
        col += sc
        for r in (10, 25):
            g = sc // r
            s = xs.reshape(g, r, DIM).sum(axis=1)
            xt[:DIM, col : col + g] = s.astype(NPDT).T
            xt[DIM, col : col + g] = float(r)
            col += g
        r0 += sc
    return np.ascontiguousarray(xt)


_NC_CACHE = {}


def _get_nc(rows):
    if rows not in _NC_CACHE:
        _NC_CACHE[rows] = build_nc(rows)
    return _NC_CACHE[rows]


def run_cores(inputs_c_e, Ws, bs, trace=False, **kw):
    """Shard, run on the 8 NeuronCores, gather. Returns (full_out, results)."""
    from concourse.bass_utils import run_bass_kernel_spmd

    x = np.ascontiguousarray(np.asarray(inputs_c_e, np.float32))
    n = x.shape[0]
    assert n == N_TOTAL
    wt = make_wt(Ws, bs).astype(NPDT)
    nc = _get_nc(R_CORE)
    in_maps = [
        {"xt": make_xt(x[c * R_CORE : (c + 1) * R_CORE]), "wt": wt}
        for c in range(N_CORES)
    ]
    res = run_bass_kernel_spmd(nc, in_maps, list(range(N_CORES)), trace=trace, **kw)
    full = np.empty((len(SCALES) * n, DIM), np.float32)
    for si in range(len(SCALES)):
        for c in range(N_CORES):
            full[si * n + c * R_CORE : si * n + (c + 1) * R_CORE] = res.results[c][
                "out"
            ][si * R_CORE : (si + 1) * R_CORE]
    return full, res


def kernel(inputs_c_e, Ws, bs):
    full, _ = run_cores(inputs_c_e, Ws, bs)
    return full
